# revision 24
# baseline (speedup 1.0000x reference)
"""DSVF kernel for trn2 v2: biquad SVF == exact causal 64-tap FIR
(poles |z|=0.426), computed as chunk-Toeplitz matmuls with the DATA as
the stationary operand, so the filtered output lands directly in
natural layout (no PE transposes at all):

  y_tile_j[p, i] = sum_q A[q,i] x_chunk(128j+p)[q]
                 + sum_q B[q,i] x_chunk(128j+p-1)[q]
  = matmul(lhsT=XT[:, 128j : 128j+128],   rhs=A)   (+ PSUM accumulate)
    matmul(lhsT=XT[:, 128j-1 : 128j+127], rhs=B)

where XT[q, c] = x[c*128 + q] is produced for free by the X-bar DMA
transpose (HWDGE, 2-byte dtype, contiguous source ~350-400 GB/s).
Inputs are sent as fp16 (host-converted), A/B are fp16, PSUM f32,
output f32.  Measured end-to-end rel err ~5e-4 vs the fp32 reference
(gate 2e-2).

Per core: 8 rows of 262144.  Per row: 1 transpose-DMA in -> 16 tiles x
(2 LDW + 2 MM(N=128)) on PE (all plain matmuls -> full HAM credit) ->
PSUM->SBUF copies (DVE banks 0,1 / ACT banks 2,3) -> contiguous DMA out
(chunk c at HBM offset 512*c, 512B-aligned writes).

Engine plan: SP = transposed in-DMAs; PE = warmup + 32 LDW/MM per row;
DVE = guard memset + 2 bank copies/row; ACT = consts + 2 bank copies +
out-DMA per row.  PSUM banks: row parity picks banks 0-3 / 4-7, tile t
-> bank group t//4, slot t%4; bank group = one accumulation group
(start on first B-MM, stop on last A-MM).
"""

import os
import numpy as np

BATCH = 64
L = 262144
N_CORES = 8
ROWS = BATCH // N_CORES  # 8 rows per core
P = 128
M = L // P      # 2048 chunks per row
T = M // P      # 16 tiles per row
GUARD = 16      # fp16 cols before data; col GUARD-1 is the zero seam col
K_TAPS = 64
TRACE = os.environ.get("DSVF_TRACE", "0") == "1"
WARMUP_MM = int(os.environ.get("DSVF_WARMUP", "16"))

_cache = {}


def _taps(g_param, R_param, m_hp, m_bp, m_lp):
    """64-tap impulse response of the biquad, float64 host math."""
    g = np.tan(np.pi * (1.0 / (1.0 + np.exp(-np.float64(g_param)))) / 2.0)
    R = np.log1p(np.exp(np.float64(R_param)))
    g2 = g * g
    b = [g2 * m_lp + g * m_bp + m_hp,
         2 * g2 * m_lp - 2 * m_hp,
         g2 * m_lp - g * m_bp + m_hp]
    a = [g2 + 2 * R * g + 1, 2 * g2 - 2, g2 - 2 * R * g + 1]
    h = np.zeros(K_TAPS, np.float64)
    for n in range(K_TAPS):
        acc = 0.0
        if n < 3:
            acc += b[n]
        if n >= 1:
            acc -= a[1] * h[n - 1]
        if n >= 2:
            acc -= a[2] * h[n - 2]
        h[n] = acc / a[0]
    return h


def _toeplitz_mats(h):
    A = np.zeros((P, P), np.float32)  # A[q, m] = h[m-q]
    B = np.zeros((P, P), np.float32)  # B[q, m] = h[m-q+128]
    for q in range(P):
        for m in range(P):
            d = m - q
            if 0 <= d < K_TAPS:
                A[q, m] = h[d]
            d2 = m - q + P
            if 0 < d2 < K_TAPS:
                B[q, m] = h[d2]
    return A, B


def _build():
    import concourse.bass as bass
    import concourse.mybir as mybir
    from contextlib import ExitStack

    f32 = mybir.dt.float32
    f16 = mybir.dt.float16

    nc = bass.Bass()
    x = nc.declare_dram_parameter("x", [ROWS, L], f16, isOutput=False)
    tab = nc.declare_dram_parameter("tab", [P, 2 * P], f16, isOutput=False)
    y = nc.declare_dram_parameter("y", [ROWS, L], f16, isOutput=True)

    xv = x.rearrange("r (c q) -> r c q", q=P)          # [R, 2048, 128]
    # chunk-oct output layout: partition p of super-group J holds chunks
    # 1024J+8p..1024J+8p+7 = 1024 contiguous samples -> 2KB HBM segments
    yv = y.rearrange("r (j p i) -> r p j i", p=P, i=8 * P)  # [R, 128, 2, 1024]

    with ExitStack() as st:
        absb = st.enter_context(nc.sbuf_tensor("absb", [P, 2 * P], f16))
        xt = [st.enter_context(nc.sbuf_tensor(f"xt{i}", [P, M + GUARD], f16))
              for i in range(4)]
        ysb = [st.enter_context(nc.sbuf_tensor(f"ysb{i}", [P, M], f16))
               for i in range(2)]
        pt = [st.enter_context(nc.psum_tensor(f"pt{i}", [P, 512], f32))
              for i in range(8)]

        dCst = st.enter_context(nc.semaphore("dCst"))
        sZg = st.enter_context(nc.semaphore("sZg"))
        dC = [st.enter_context(nc.semaphore(f"dC{g}")) for g in range(4)]
        dInS = [st.enter_context(nc.semaphore(f"dInS{i}")) for i in range(4)]
        dOutP = [st.enter_context(nc.semaphore(f"dOutP{i}")) for i in range(2)]
        sMm = st.enter_context(nc.semaphore("sMm"))    # +1 per PE bank group
        sYcD = st.enter_context(nc.semaphore("sYcD"))  # +1 per DVE bank copy
        sYcA = st.enter_context(nc.semaphore("sYcA"))  # +1 per ACT bank copy

        blk = st.enter_context(nc.Block())

        @blk.sync
        def _(sp):
            # row 0 in 4 pieces (separate sems — concurrent transfers must
            # not share a sem); rows alternate parity sems
            for g in range(4):
                sp.dma_start(out=xt[0][:, GUARD + 512 * g:GUARD + 512 * (g + 1)],
                             in_=xv[0][512 * g:512 * (g + 1), :],
                             transpose=True).then_inc(dC[g], 16)
            # ALL transposes on one ring: concurrent X-bar transposes
            # (even on different HWDGE rings) corrupt each other
            for r in range(1, ROWS):
                if r >= 4:
                    sp.wait_ge(sMm, 4 * (r - 4) + 4)  # xt[r%4] LDWs done
                sp.dma_start(out=xt[r % 4][:, GUARD:GUARD + M],
                             in_=xv[r],
                             transpose=True).then_inc(dInS[r % 4], 16)

        @blk.tensor
        def _(pe):
            pe.wait_ge(dCst, 16)
            pe.wait_ge(sZg, 1)
            # HAM warmup while row 0 streams in (plain matmuls on consts)
            for i in range(WARMUP_MM):
                pe.matmul(pt[7][:, 0:P], absb[:, 0:P], absb[:, P:2 * P],
                          start=(i == 0), stop=(i == WARMUP_MM - 1))
            xt8 = [xt[i].rearrange("q (c eight) -> q eight c", eight=8)
                   for i in range(4)]
            for r in range(ROWS):
                x8 = xt8[r % 4]
                for g in range(4):
                    J, b = g // 2, g % 2
                    bank = pt[4 * (r % 2) + g]
                    if r == 0:
                        if b == 0:
                            pe.wait_ge(dC[2 * J], 16)
                            pe.wait_ge(dC[2 * J + 1], 16)
                    elif g == 0:
                        pe.wait_ge(dInS[r % 4], 16 * ((r - 1) // 4 + 1))
                    if r >= 2:
                        if g < 2:
                            pe.wait_ge(sYcD, 2 * (r - 2) + g + 1)
                        else:
                            pe.wait_ge(sYcA, 2 * (r - 2) + (g - 2) + 1)
                    for kk in range(4):
                        k = 4 * b + kk
                        # class k: chunks 1024J+8p+k at partition p
                        if k == 0:
                            lhsB = x8[:, 7, 1 + 128 * J:129 + 128 * J]
                        else:
                            lhsB = x8[:, k - 1, 2 + 128 * J:130 + 128 * J]
                        lhsA = x8[:, k, 2 + 128 * J:130 + 128 * J]
                        pe.matmul(bank[:, kk * P:(kk + 1) * P], lhsB,
                                  absb[:, P:2 * P],
                                  start=(kk == 0), stop=False)
                        ins = pe.matmul(bank[:, kk * P:(kk + 1) * P], lhsA,
                                        absb[:, 0:P],
                                        start=False, stop=(kk == 3))
                    # cheap engine-completion tick; copiers wait one tick
                    # LATER (+1 group ≈ 600ns) to cover the PSUM drain —
                    # an explicit pe.drain() here costs ~690ns each
                    ins.then_inc(sMm, 1)
            # final tick so the last copies' +1-lag waits can be satisfied;
            # drain also guarantees row 7 group 3's PSUM writes landed
            pe.drain().then_inc(sMm, 1)

        @blk.vector
        def _(dve):
            for i in range(3):
                dve.memset(xt[i][:, 0:GUARD], 0.0)
            dve.memset(xt[3][:, 0:GUARD], 0.0).then_inc(sZg, 1)
            for r in range(ROWS):
                if r >= 2:
                    dve.wait_ge(dOutP[r % 2], 16 * (r // 2))  # ysb WAR
                for g in range(2):
                    dve.wait_ge(sMm, 4 * r + g + 2)  # +1 tick: PSUM drain
                    dve.tensor_copy(ysb[r % 2][:, 512 * g:512 * (g + 1)],
                                    pt[4 * (r % 2) + g][:])
                    dve.drain().then_inc(sYcD, 1)

        @blk.scalar
        def _(act):
            act.dma_start(out=absb[:], in_=tab[:]).then_inc(dCst, 16)
            for r in range(ROWS):
                if r >= 2:
                    act.wait_ge(dOutP[r % 2], 16 * (r // 2))  # ysb WAR
                for g in range(2, 4):
                    act.wait_ge(sMm, 4 * r + g + 2)  # +1 tick: PSUM drain
                    act.copy(out=ysb[r % 2][:, 512 * g:512 * (g + 1)],
                             in_=pt[4 * (r % 2) + g][:]).then_inc(sYcA, 1)
                act.wait_ge(sYcD, 2 * r + 2)
                act.dma_start(out=yv[r], in_=ysb[r % 2][:]
                              ).then_inc(dOutP[r % 2], 16)
            act.wait_ge(dOutP[0], 64)
            act.wait_ge(dOutP[1], 64)

    return nc


def _get_nc():
    if "nc" not in _cache:
        _cache["nc"] = _build()
    return _cache["nc"]


def kernel(**inputs):
    from concourse.bass_utils import run_bass_kernel_spmd

    x = np.asarray(inputs["x"], dtype=np.float32)
    assert x.shape == (BATCH, L), x.shape
    x16 = np.ascontiguousarray(x.astype(np.float16))
    h = _taps(float(np.asarray(inputs["g_param"]).reshape(-1)[0]),
              float(np.asarray(inputs["R_param"]).reshape(-1)[0]),
              float(np.asarray(inputs["m_hp"]).reshape(-1)[0]),
              float(np.asarray(inputs["m_bp"]).reshape(-1)[0]),
              float(np.asarray(inputs["m_lp"]).reshape(-1)[0]))
    A, B = _toeplitz_mats(h)
    tab = np.concatenate([A, B], axis=1).astype(np.float16)

    nc = _get_nc()
    core_ids = list(range(N_CORES))
    in_maps = [
        {"x": x16[i * ROWS:(i + 1) * ROWS], "tab": tab}
        for i in range(N_CORES)
    ]
    kwargs = {}
    if TRACE:
        kwargs["tmpdir"] = os.environ.get("DSVF_TRACE_DIR") or None
    res = run_bass_kernel_spmd(nc, in_maps, core_ids, trace=TRACE, **kwargs)
    if TRACE:
        kernel.last_exec_time_ns = res.exec_time_ns
        kernel.last_results = res
    out = np.concatenate([np.asarray(res.results[i]["y"], dtype=np.float32)
                          for i in range(N_CORES)], axis=0)
    return out


kernel.last_exec_time_ns = None


# revision 26
# speedup vs baseline: 1.0224x; 1.0224x over previous
"""DSVF kernel for trn2 v2: biquad SVF == exact causal 64-tap FIR
(poles |z|=0.426), computed as chunk-Toeplitz matmuls with the DATA as
the stationary operand, so the filtered output lands directly in
natural layout (no PE transposes at all):

  y_tile_j[p, i] = sum_q A[q,i] x_chunk(128j+p)[q]
                 + sum_q B[q,i] x_chunk(128j+p-1)[q]
  = matmul(lhsT=XT[:, 128j : 128j+128],   rhs=A)   (+ PSUM accumulate)
    matmul(lhsT=XT[:, 128j-1 : 128j+127], rhs=B)

where XT[q, c] = x[c*128 + q] is produced for free by the X-bar DMA
transpose (HWDGE, 2-byte dtype, contiguous source ~350-400 GB/s).
Inputs are sent as fp16 (host-converted), A/B are fp16, PSUM f32,
output f32.  Measured end-to-end rel err ~5e-4 vs the fp32 reference
(gate 2e-2).

Per core: 8 rows of 262144.  Per row: 1 transpose-DMA in -> 16 tiles x
(2 LDW + 2 MM(N=128)) on PE (all plain matmuls -> full HAM credit) ->
PSUM->SBUF copies (DVE banks 0,1 / ACT banks 2,3) -> contiguous DMA out
(chunk c at HBM offset 512*c, 512B-aligned writes).

Engine plan: SP = transposed in-DMAs; PE = warmup + 32 LDW/MM per row;
DVE = guard memset + 2 bank copies/row; ACT = consts + 2 bank copies +
out-DMA per row.  PSUM banks: row parity picks banks 0-3 / 4-7, tile t
-> bank group t//4, slot t%4; bank group = one accumulation group
(start on first B-MM, stop on last A-MM).
"""

import os
import numpy as np

BATCH = 64
L = 262144
N_CORES = 8
ROWS = BATCH // N_CORES  # 8 rows per core
P = 128
M = L // P      # 2048 chunks per row
T = M // P      # 16 tiles per row
GUARD = 16      # fp16 cols before data; col GUARD-1 is the zero seam col
K_TAPS = 64
TRACE = os.environ.get("DSVF_TRACE", "0") == "1"
WARMUP_MM = int(os.environ.get("DSVF_WARMUP", "16"))

_cache = {}


def _taps(g_param, R_param, m_hp, m_bp, m_lp):
    """64-tap impulse response of the biquad, float64 host math."""
    g = np.tan(np.pi * (1.0 / (1.0 + np.exp(-np.float64(g_param)))) / 2.0)
    R = np.log1p(np.exp(np.float64(R_param)))
    g2 = g * g
    b = [g2 * m_lp + g * m_bp + m_hp,
         2 * g2 * m_lp - 2 * m_hp,
         g2 * m_lp - g * m_bp + m_hp]
    a = [g2 + 2 * R * g + 1, 2 * g2 - 2, g2 - 2 * R * g + 1]
    h = np.zeros(K_TAPS, np.float64)
    for n in range(K_TAPS):
        acc = 0.0
        if n < 3:
            acc += b[n]
        if n >= 1:
            acc -= a[1] * h[n - 1]
        if n >= 2:
            acc -= a[2] * h[n - 2]
        h[n] = acc / a[0]
    return h


def _toeplitz_mats(h):
    A = np.zeros((P, P), np.float32)  # A[q, m] = h[m-q]
    B = np.zeros((P, P), np.float32)  # B[q, m] = h[m-q+128]
    for q in range(P):
        for m in range(P):
            d = m - q
            if 0 <= d < K_TAPS:
                A[q, m] = h[d]
            d2 = m - q + P
            if 0 < d2 < K_TAPS:
                B[q, m] = h[d2]
    return A, B


def _build():
    import concourse.bass as bass
    import concourse.mybir as mybir
    from contextlib import ExitStack

    f32 = mybir.dt.float32
    f16 = mybir.dt.float16

    nc = bass.Bass()
    x = nc.declare_dram_parameter("x", [ROWS, L], f16, isOutput=False)
    tab = nc.declare_dram_parameter("tab", [P, 2 * P], f16, isOutput=False)
    y = nc.declare_dram_parameter("y", [ROWS, L], f16, isOutput=True)

    xv = x.rearrange("r (c q) -> r c q", q=P)          # [R, 2048, 128]
    # 16-class layout: class k = chunks {16p+k}, so partition p ends up
    # holding its own contiguous 2048-sample segment -> the out-DMA is a
    # plain contiguous [128, 2048] store (4KB per partition, full rate)
    yv = y.rearrange("r (p m) -> r p m", p=P)          # [R, 128, 2048]
    # row 0 uses the oct layout (chunks 1024J+8p+..) so PE consumption can
    # start after half the row arrived and trails the X-bar retirement
    yv8 = y.rearrange("r (j p i) -> r p j i", p=P, i=8 * P)  # [R,128,2,1024]

    with ExitStack() as st:
        absb = st.enter_context(nc.sbuf_tensor("absb", [P, 2 * P], f16))
        xt = [st.enter_context(nc.sbuf_tensor(f"xt{i}", [P, M + GUARD], f16))
              for i in range(4)]
        ysb = [st.enter_context(nc.sbuf_tensor(f"ysb{i}", [P, M], f16))
               for i in range(2)]
        pt = [st.enter_context(nc.psum_tensor(f"pt{i}", [P, 512], f32))
              for i in range(8)]

        dCst = st.enter_context(nc.semaphore("dCst"))
        sZg = st.enter_context(nc.semaphore("sZg"))
        dC = [st.enter_context(nc.semaphore(f"dC{g}")) for g in range(4)]
        dInS = [st.enter_context(nc.semaphore(f"dInS{i}")) for i in range(4)]
        dOutP = [st.enter_context(nc.semaphore(f"dOutP{i}")) for i in range(2)]
        sMm = st.enter_context(nc.semaphore("sMm"))    # +1 per PE bank group
        sYcD = st.enter_context(nc.semaphore("sYcD"))  # +1 per DVE bank copy
        sYcA = st.enter_context(nc.semaphore("sYcA"))  # +1 per ACT bank copy

        blk = st.enter_context(nc.Block())

        @blk.sync
        def _(sp):
            for g in range(4):
                sp.dma_start(out=xt[0][:, GUARD + 512 * g:GUARD + 512 * (g + 1)],
                             in_=xv[0][512 * g:512 * (g + 1), :],
                             transpose=True).then_inc(dC[g], 16)
            # ALL transposes on one ring: concurrent X-bar transposes
            # (even on different HWDGE rings) corrupt each other
            for r in range(1, ROWS):
                if r >= 4:
                    sp.wait_ge(sMm, 4 * (r - 4) + 4)  # xt[r%4] LDWs done
                sp.dma_start(out=xt[r % 4][:, GUARD:GUARD + M],
                             in_=xv[r],
                             transpose=True).then_inc(dInS[r % 4], 16)

        @blk.tensor
        def _(pe):
            pe.wait_ge(dCst, 16)
            pe.wait_ge(sZg, 1)
            # HAM warmup while row 0 streams in (plain matmuls on consts)
            for i in range(WARMUP_MM):
                pe.matmul(pt[7][:, 0:P], absb[:, 0:P], absb[:, P:2 * P],
                          start=(i == 0), stop=(i == WARMUP_MM - 1))
            xt16 = [xt[i].rearrange("q (c s) -> q s c", s=16)
                    for i in range(4)]
            xt8v = xt[0].rearrange("q (c eight) -> q eight c", eight=8)
            for r in range(ROWS):
                x16 = xt16[r % 4]
                for g in range(4):
                    bank = pt[4 * (r % 2) + g]
                    if r == 0:
                        if g % 2 == 0:
                            pe.wait_ge(dC[g], 16)
                            pe.wait_ge(dC[g + 1], 16)
                    elif g == 0:
                        pe.wait_ge(dInS[r % 4], 16 * ((r - 1) // 4 + 1))
                    if r >= 2:
                        if g < 2:
                            pe.wait_ge(sYcD, 2 * (r - 2) + g + 1)
                        else:
                            pe.wait_ge(sYcA, 2 * (r - 2) + (g - 2) + 1)
                    for kk in range(4):
                        if r == 0:
                            J, k8 = g // 2, 4 * (g % 2) + kk
                            if k8 == 0:
                                lhsB = xt8v[:, 7, 1 + 128 * J:129 + 128 * J]
                            else:
                                lhsB = xt8v[:, k8 - 1, 2 + 128 * J:130 + 128 * J]
                            lhsA = xt8v[:, k8, 2 + 128 * J:130 + 128 * J]
                            pe.matmul(bank[:, kk * P:(kk + 1) * P], lhsB,
                                      absb[:, P:2 * P],
                                      start=(kk == 0), stop=False)
                            ins = pe.matmul(bank[:, kk * P:(kk + 1) * P], lhsA,
                                            absb[:, 0:P],
                                            start=False, stop=(kk == 3))
                            continue
                        k = 4 * g + kk
                        # class k: chunks 16p+k at partition p
                        if k == 0:
                            lhsB = x16[:, 15, 0:128]
                        else:
                            lhsB = x16[:, k - 1, 1:129]
                        lhsA = x16[:, k, 1:129]
                        pe.matmul(bank[:, kk * P:(kk + 1) * P], lhsB,
                                  absb[:, P:2 * P],
                                  start=(kk == 0), stop=False)
                        ins = pe.matmul(bank[:, kk * P:(kk + 1) * P], lhsA,
                                        absb[:, 0:P],
                                        start=False, stop=(kk == 3))
                    # cheap engine-completion tick; copiers wait one tick
                    # LATER (+1 group ≈ 600ns) to cover the PSUM drain —
                    # an explicit pe.drain() here costs ~690ns each
                    ins.then_inc(sMm, 1)
            # final tick so the last copies' +1-lag waits can be satisfied;
            # drain also guarantees row 7 group 3's PSUM writes landed
            pe.drain().then_inc(sMm, 1)

        @blk.vector
        def _(dve):
            for i in range(3):
                dve.memset(xt[i][:, 0:GUARD], 0.0)
            dve.memset(xt[3][:, 0:GUARD], 0.0).then_inc(sZg, 1)
            for r in range(ROWS):
                if r >= 2:
                    dve.wait_ge(dOutP[r % 2], 16 * (r // 2))  # ysb WAR
                for g in range(2):
                    dve.wait_ge(sMm, 4 * r + g + 2)  # +1 tick: PSUM drain
                    dve.tensor_copy(ysb[r % 2][:, 512 * g:512 * (g + 1)],
                                    pt[4 * (r % 2) + g][:])
                    dve.drain().then_inc(sYcD, 1)

        @blk.scalar
        def _(act):
            act.dma_start(out=absb[:], in_=tab[:]).then_inc(dCst, 16)
            for r in range(ROWS):
                if r >= 2:
                    act.wait_ge(dOutP[r % 2], 16 * (r // 2))  # ysb WAR
                for g in range(2, 4):
                    act.wait_ge(sMm, 4 * r + g + 2)  # +1 tick: PSUM drain
                    act.copy(out=ysb[r % 2][:, 512 * g:512 * (g + 1)],
                             in_=pt[4 * (r % 2) + g][:]).then_inc(sYcA, 1)
                act.wait_ge(sYcD, 2 * r + 2)
                act.dma_start(out=(yv8[0] if r == 0 else yv[r]),
                              in_=ysb[r % 2][:]
                              ).then_inc(dOutP[r % 2], 16)
            act.wait_ge(dOutP[0], 64)
            act.wait_ge(dOutP[1], 64)

    return nc


def _get_nc():
    if "nc" not in _cache:
        _cache["nc"] = _build()
    return _cache["nc"]


def kernel(**inputs):
    from concourse.bass_utils import run_bass_kernel_spmd

    x = np.asarray(inputs["x"], dtype=np.float32)
    assert x.shape == (BATCH, L), x.shape
    x16 = np.ascontiguousarray(x.astype(np.float16))
    h = _taps(float(np.asarray(inputs["g_param"]).reshape(-1)[0]),
              float(np.asarray(inputs["R_param"]).reshape(-1)[0]),
              float(np.asarray(inputs["m_hp"]).reshape(-1)[0]),
              float(np.asarray(inputs["m_bp"]).reshape(-1)[0]),
              float(np.asarray(inputs["m_lp"]).reshape(-1)[0]))
    A, B = _toeplitz_mats(h)
    tab = np.concatenate([A, B], axis=1).astype(np.float16)

    nc = _get_nc()
    core_ids = list(range(N_CORES))
    in_maps = [
        {"x": x16[i * ROWS:(i + 1) * ROWS], "tab": tab}
        for i in range(N_CORES)
    ]
    kwargs = {}
    if TRACE:
        kwargs["tmpdir"] = os.environ.get("DSVF_TRACE_DIR") or None
    res = run_bass_kernel_spmd(nc, in_maps, core_ids, trace=TRACE, **kwargs)
    if TRACE:
        kernel.last_exec_time_ns = res.exec_time_ns
        kernel.last_results = res
    out = np.concatenate([np.asarray(res.results[i]["y"], dtype=np.float32)
                          for i in range(N_CORES)], axis=0)
    return out


kernel.last_exec_time_ns = None
